# revision 1
# baseline (speedup 1.0000x reference)
"""GCN (PyG GCNConv) forward on 8 Trainium2 NeuronCores.

Reference computes z = D^-1/2 (A+I) D^-1/2 (X @ W2) + b2  (conv1 is dead code,
its result is never used).

Strategy (1D destination partition, standard distributed GCN):
  * Host: compute degrees + symmetric normalization, fold isd[src] into X,
    partition messages (edges + self loops) by destination shard
    (8 cores x 6250 nodes), build degree-sorted padded-CSR slot grids and
    int16 gather indices. The Y table is split in two halves so table-local
    indices fit in int16 (dma_gather limit: table < 32768 rows).
  * Device (identical program on all 8 cores, per-core data via in_maps):
      phase 1: GEMM  Y = XT.T @ W2  (XT pre-scaled by isd[src]) written to
               two DRAM half tables, [128, 197, 64] each.
      phase 2: pass A: dma_gather message rows from half A (messages sorted
               by dst, padded to per-tile max degree), DVE pairwise-tree
               segmented sum, partial written to DRAM. Gathers of pass A
               overlap with the half-B GEMM.
      phase 3: pass B: same gathers from half B, plus a crossmap gather of
               the pass-A partial rows (the two passes use different
               degree-sorted dst orders), combine, scale by isd[dst], out.
  * Host: inverse-permute per-core outputs into the global row order, + b2.
"""

import numpy as np

import concourse.bacc as bacc
import concourse.bass as bass
import concourse.mybir as mybir
from concourse.bass_utils import run_bass_kernel_spmd
from concourse.library_config import mlp

# ---------------- problem constants (hardcoded per contract) ----------------
N = 50000          # nodes
FIN = 128          # input channels
FOUT = 64          # output channels
NCORES = 8
PER = N // NCORES  # 6250 dst nodes per core
TILES = 49         # ceil(PER/128)
PADN = TILES * 128  # 6272 padded dst slots per core
HC = 197           # Y-table chunks (of 128 rows) per half
HALFC = HC * 128   # 25216 table rows per half
NCOL = 2 * HALFC   # 50432 XT columns (with zero pads)
ZSLOT = 40 * HC + 195  # table idx of a guaranteed-zero row (col 25000/50216)
CAP = 6144         # max num_idxs per reduce batch (48 chunk-layers)
CH_T = 8           # GEMM tiles per chunk (1024 XT cols per DMA)

_cache = {}


# ------------------------------ host schedule -------------------------------
def _node_table(n):
    """node id -> (half, table idx). Vectorized."""
    col = n + 216 * (n >= 25000)
    c = col // 128
    p = col % 128
    half = (c >= HC).astype(np.int64)
    idx = p * HC + (c - HC * half)
    return half, idx


def _wrap_idx(flat):
    """[L] int16 slot list -> [128, L//16] wrapped+replicated index array."""
    L = flat.shape[0]
    assert L % 16 == 0
    a16 = flat.reshape(L // 16, 16).T  # idx i at [i%16, i//16]
    return np.ascontiguousarray(np.tile(a16, (8, 1)))


def _build_schedule(src, dst):
    """Returns (isd, shared batch structure, per-core index arrays, outmaps)."""
    msrc = np.concatenate([src, np.arange(N, dtype=np.int64)])
    mdst = np.concatenate([dst, np.arange(N, dtype=np.int64)])
    deg = np.bincount(mdst, minlength=N)
    isd = (1.0 / np.sqrt(np.maximum(deg, 1))).astype(np.float32)

    mhalf, mtab = _node_table(msrc)
    core = mdst // PER
    dloc = mdst - core * PER

    percore = []
    Dmax = np.zeros((2, TILES), dtype=np.int64)
    for k in range(NCORES):
        entry = {}
        for h in (0, 1):
            sel = (core == k) & (mhalf == h)
            d = dloc[sel]
            t = mtab[sel]
            cnt = np.bincount(d, minlength=PER)
            order = np.argsort(cnt, kind="stable")  # ascending degree
            pos = np.empty(PER, dtype=np.int64)
            pos[order] = np.arange(PER) + (PADN - PER)  # dummies at 0..21
            o2 = np.argsort(d, kind="stable")
            ds = d[o2]
            starts = np.searchsorted(ds, np.arange(PER))
            j = np.arange(ds.shape[0]) - starts[ds]  # rank within dst
            mpos = pos[ds]
            cntpad = np.zeros(PADN, dtype=np.int64)
            cntpad[pos] = cnt
            Dmax[h] = np.maximum(Dmax[h], cntpad.reshape(TILES, 128).max(axis=1))
            entry[h] = dict(pos=pos, tile=mpos // 128, pslot=mpos % 128,
                            j=j, tab=t[o2])
        percore.append(entry)

    # shared batches per pass: consecutive tiles padded to the batch max
    # degree; extend while the padding this adds stays small and the
    # reduce-buffer cap is respected
    batches = {0: [], 1: []}
    for h in (0, 1):
        t0 = 0
        while t0 < TILES:
            g = 1
            dbatch = max(1, int(Dmax[h][t0]))
            while t0 + g < TILES:
                nd = max(dbatch, int(Dmax[h][t0 + g]))
                if nd * (g + 1) > CAP // 128:
                    break
                waste = nd * (g + 1) - (
                    dbatch * g + max(1, int(Dmax[h][t0 + g])))
                if waste > max(2, (nd * (g + 1)) // 16):
                    break
                dbatch = nd
                g += 1
            batches[h].append((t0, g, dbatch))
            t0 += g

    inmaps = []
    outmaps = []
    for k in range(NCORES):
        m = {}
        for h in (0, 1):
            e = percore[k][h]
            grids = []
            for (t0, g, db) in batches[h]:
                grid = np.full(128 * db * g, ZSLOT, dtype=np.int16)
                sel = (e["tile"] >= t0) & (e["tile"] < t0 + g)
                gg = e["tile"][sel] - t0
                lin = (gg * db + e["j"][sel]) * 128 + e["pslot"][sel]
                grid[lin] = e["tab"][sel].astype(np.int16)
                grids.append(grid)
            m["idx" + "ab"[h]] = _wrap_idx(np.concatenate(grids))
        posA = percore[k][0]["pos"]
        posB = percore[k][1]["pos"]
        xm = np.arange(PADN, dtype=np.int16)  # dummies map to themselves
        xm[posB] = posA.astype(np.int16)
        m["xmap"] = _wrap_idx(xm)
        isdb = np.zeros(PADN, dtype=np.float32)
        isdb[posB] = isd[k * PER:(k + 1) * PER]
        m["isdb"] = np.ascontiguousarray(isdb.reshape(TILES, 128).T)
        inmaps.append(m)
        om = np.full(PADN, -1, dtype=np.int64)
        om[posB] = np.arange(k * PER, (k + 1) * PER)
        outmaps.append(om)

    return isd, batches, inmaps, outmaps


# ------------------------------ device program ------------------------------
NQ = 4        # SWDGE queues (parallel Q7 descriptor generation)
NBUF = 4      # gather buffer rotation depth
PIECE_CH = 8  # chunk-layers per dma_gather piece (ring carveout limit)
CAPCH = 48    # max chunk-layers (G*D) per batch -> <= 6 pieces


def _build_program(batches, reps=1, mode="full"):
    nc = bacc.Bacc("TRN2", debug=False, num_swdge_queues=NQ)
    f32 = mybir.dt.float32
    i16 = mybir.dt.int16

    colsA = sum(128 * db * g for (_, g, db) in batches[0]) // 16
    colsB = sum(128 * db * g for (_, g, db) in batches[1]) // 16
    dgmax = max(db * g for h in (0, 1) for (_, g, db) in batches[h])
    MAXP = -(-dgmax // PIECE_CH)

    xt = nc.declare_dram_parameter("xt", [FIN, NCOL], f32, isOutput=False)
    w = nc.declare_dram_parameter("w", [FIN, FOUT], f32, isOutput=False)
    idxa = nc.declare_dram_parameter("idxa", [128, colsA], i16, isOutput=False)
    idxb = nc.declare_dram_parameter("idxb", [128, colsB], i16, isOutput=False)
    xmap = nc.declare_dram_parameter("xmap", [128, PADN // 16], i16, isOutput=False)
    isdb = nc.declare_dram_parameter("isdb", [128, TILES], f32, isOutput=False)
    out = nc.declare_dram_parameter("out", [PADN, FOUT], f32, isOutput=True)

    ya = nc.dram_tensor("ya", [128, HC, FOUT], f32)
    yb = nc.dram_tensor("yb", [128, HC, FOUT], f32)
    pa = nc.dram_tensor("pa", [PADN, FOUT], f32)
    yh = {0: ya, 1: yb}

    # per-body GEMM chunk list: (half, tile offset within half, ntiles)
    chunks1 = []
    for h in (0, 1):
        c0 = 0
        while c0 < HC:
            nt = min(CH_T, HC - c0)
            chunks1.append((h, c0, nt))
            c0 += nt
    NCH = len(chunks1)
    assert NCH % 2 == 0
    # per-body gather batch list: (pass, t0, g, db, idx col offset)
    gb1 = []
    for h in (0, 1):
        off = 0
        for (t0, g, db) in batches[h]:
            assert db * g <= CAPCH
            gb1.append((h, t0, g, db, off))
            off += db * g * 8
    NBAT = len(gb1)
    NA = len(batches[0])
    NXP = -(-TILES // PIECE_CH)

    def npieces(db, g):
        return -(-(db * g) // PIECE_CH)

    # global (repeat-extended) bookkeeping
    GCH = [(r, ci) + chunks1[ci] for r in range(reps) for ci in range(NCH)]
    GBT = [(r, bi) + gb1[bi] for r in range(reps) for bi in range(NBAT)]
    ywr_cum = [0, 0]
    ywr_after = []
    for gci in range(len(GCH)):
        ywr_cum[gci % 2] += 16
        ywr_after.append(tuple(ywr_cum))

    def dve_ops(db, h):
        n = 0
        cur = db
        while cur > 1:
            n += 1
            cur = cur - cur // 2
        n += 2 if h == 1 else 1   # combine+scale (B) / move-to-res (A)
        return n

    # Global piece sequence: pieces strictly alternate SWDGE queues so the
    # two descriptor rings ping-pong (a queue's next piece must wait for its
    # previous piece's full drain). Completion sems rotate per queue with
    # depth KK so each sem has at most one outstanding DMA.
    KK = 16
    vch_cum = 0
    vch_after = []
    pc_body = [0]  # body-local piece counter (resets per repeat)

    def alloc_piece(plist):
        gp = pc_body[0]
        pc_body[0] += 1
        q = gp % NQ
        kk = (gp // NQ) % KK
        plist.append((q, kk))
        return q, kk

    batch_pieces1 = []   # per body-local batch: [(q, kk), ...]
    xmap_pieces1 = []    # crossmap pieces
    for bi, (h, t0, g, db, off) in enumerate(gb1):
        pl = []
        for _ in range(npieces(db, g)):
            alloc_piece(pl)
        batch_pieces1.append(pl)
        if bi == NA - 1:
            for _ in range(NXP):
                alloc_piece(xmap_pieces1)
    # sem values: cumulative uses across the repeat-extended sequence
    use_cum = {}
    batch_vals = []      # per global batch: [(q, kk, val), ...]
    xmap_vals = []       # per repeat: [(q, kk, val), ...]
    for r in range(reps):
        for bi in range(NBAT):
            vals = []
            for (q, kk) in batch_pieces1[bi]:
                use_cum[(q, kk)] = use_cum.get((q, kk), 0) + 16
                vals.append((q, kk, use_cum[(q, kk)]))
            batch_vals.append(vals)
            if bi == NA - 1:
                xv = []
                for (q, kk) in xmap_pieces1:
                    use_cum[(q, kk)] = use_cum.get((q, kk), 0) + 16
                    xv.append((q, kk, use_cum[(q, kk)]))
                xmap_vals.append(xv)
    rep_end_vals = []    # all-sem snapshot at end of each repeat
    cum2 = {}
    for r in range(reps):
        for bi in range(NBAT):
            for (q, kk, val) in batch_vals[r * NBAT + bi]:
                cum2[(q, kk)] = val
            if bi == NA - 1:
                for (q, kk, val) in xmap_vals[r]:
                    cum2[(q, kk)] = val
        rep_end_vals.append(dict(cum2))
    for gbi, (r, bi, h, t0, g, db, off) in enumerate(GBT):
        vch_cum += dve_ops(db, h)
        vch_after.append(vch_cum)

    from contextlib import ExitStack
    with ExitStack() as ctx:
        w_sb = ctx.enter_context(nc.sbuf_tensor("w_sb", [FIN, FOUT], f32))
        xt_sb = ctx.enter_context(
            nc.sbuf_tensor("xt_sb", [FIN, 2, CH_T * 128], f32))
        y_sb = ctx.enter_context(
            nc.sbuf_tensor("y_sb", [128, 2, CH_T, FOUT], f32))
        ia_sb = ctx.enter_context(nc.sbuf_tensor("ia_sb", [128, colsA], i16))
        ib_sb = ctx.enter_context(nc.sbuf_tensor("ib_sb", [128, colsB], i16))
        xm_sb = ctx.enter_context(
            nc.sbuf_tensor("xm_sb", [128, PADN // 16], i16))
        isdb_sb = ctx.enter_context(
            nc.sbuf_tensor("isdb_sb", [128, TILES], f32))
        g_sb = ctx.enter_context(
            nc.sbuf_tensor("g_sb", [128, NBUF, dgmax, FOUT], f32))
        x_sb = ctx.enter_context(
            nc.sbuf_tensor("x_sb", [128, TILES, FOUT], f32))
        res_sb = ctx.enter_context(
            nc.sbuf_tensor("res_sb", [128, TILES, FOUT], f32))
        lib_scr = ctx.enter_context(nc.sbuf_tensor("lib_scr", [128, 4], i16))
        ps = ctx.enter_context(
            nc.psum_tensor("ps", [128, 2 * CH_T * FOUT], f32))
        names = (["LIB", "PRM", "XTL0", "XTL1", "MMC", "YCP", "YWR0", "YWR1",
                  "VCH", "BWPA", "BWOUT"]
                 + ["GS%d_%d" % (q, kk) for q in range(NQ) for kk in range(KK)])
        sem = {n: ctx.enter_context(nc.semaphore(n)) for n in names}
        LIB, PRM, MMC, YCP, VCH = (
            sem["LIB"], sem["PRM"], sem["MMC"], sem["YCP"], sem["VCH"])
        BWPA, BWOUT = sem["BWPA"], sem["BWOUT"]
        XTL = [sem["XTL0"], sem["XTL1"]]
        YWR = [sem["YWR0"], sem["YWR1"]]
        GS = [[sem["GS%d_%d" % (q, kk)] for kk in range(KK)]
              for q in range(NQ)]
        block = ctx.enter_context(nc.Block())

        def ywr_gate(r, h):
            gci = r * NCH + (NCH // 2 - 1 if h == 0 else NCH - 1)
            return ywr_after[gci]

        @block.gpsimd
        def _(g: bass.BassGpSimd):
            g.load_library(mlp)
            # dummy SWDGE DMA: its completion implies the ucode reload is
            # fully done (HWDGE transfers during the reload crash the device)
            g.dma_start(lib_scr[:], xmap[:, :4]).then_inc(LIB, 16)
            if mode == "gemm":
                return
            for gbi, (r, bi, h, t0, gt, db, off) in enumerate(GBT):
                if gbi == 0:
                    g.wait_ge(PRM, 16 * 5)
                if mode == "full":
                    gate = ywr_gate(r, h)
                    g.wait_ge(YWR[0], gate[0])
                    g.wait_ge(YWR[1], gate[1])
                if gbi >= NBUF and mode != "gatheronly":
                    g.wait_ge(VCH, vch_after[gbi - NBUF])  # buf free
                isb = ia_sb if h == 0 else ib_sb
                nch = db * gt
                for j, j0 in enumerate(range(0, nch, PIECE_CH)):
                    nj = min(PIECE_CH, nch - j0)
                    q, kk, _ = batch_vals[gbi][j]
                    g.dma_gather(
                        g_sb[:, gbi % NBUF, j0: j0 + nj, :],
                        yh[h][:].rearrange("p c f -> (p c) f"),
                        isb[:, off + j0 * 8: off + j0 * 8 + nj * 8],
                        128 * nj, 128 * nj, FOUT,
                        queue_num=q,
                    ).then_inc(GS[q][kk], 16)
                if bi == NA - 1:
                    # crossmap gather of this repeat's pass-A partial rows
                    if mode != "gatheronly":
                        g.wait_ge(BWPA, 16 * (r + 1))
                    for j, j0 in enumerate(range(0, TILES, PIECE_CH)):
                        nj = min(PIECE_CH, TILES - j0)
                        q, kk, _ = xmap_vals[r][j]
                        g.dma_gather(
                            x_sb[:, j0: j0 + nj, :], pa[:],
                            xm_sb[:, j0 * 8: j0 * 8 + nj * 8],
                            128 * nj, 128 * nj, FOUT,
                            queue_num=q,
                        ).then_inc(GS[q][kk], 16)

        def _emit_gemm_dmas(s, r):
            for ci in range(NCH):
                gci = r * NCH + ci
                h, c0, nt = chunks1[ci]
                if gci >= 2:
                    s.wait_ge(MMC, gci - 1)  # xt buf parity free
                col0 = (h * HC + c0) * 128
                s.dma_start(
                    xt_sb[:, gci % 2, : nt * 128],
                    xt[:, col0: col0 + nt * 128],
                ).then_inc(XTL[gci % 2], 16)
                if ci >= 1:
                    pg = gci - 1
                    ph, pc0, pnt = chunks1[ci - 1]
                    s.wait_ge(YCP, pg + 1)
                    s.dma_start(
                        yh[ph][:, pc0: pc0 + pnt, :],
                        y_sb[:, pg % 2, :pnt, :],
                    ).then_inc(YWR[pg % 2], 16)
            gci = r * NCH + NCH - 1
            lh, lc0, lnt = chunks1[NCH - 1]
            s.wait_ge(YCP, gci + 1)
            s.dma_start(
                yh[lh][:, lc0: lc0 + lnt, :], y_sb[:, gci % 2, :lnt, :]
            ).then_inc(YWR[gci % 2], 16)

        @block.sync
        def _(s: bass.BassEngine):
            s.wait_ge(LIB, 16)
            s.dma_start(w_sb[:], w[:]).then_inc(PRM, 16)
            s.dma_start(ia_sb[:], idxa[:]).then_inc(PRM, 16)
            s.dma_start(ib_sb[:], idxb[:]).then_inc(PRM, 16)
            s.dma_start(xm_sb[:], xmap[:]).then_inc(PRM, 16)
            s.dma_start(isdb_sb[:], isdb[:]).then_inc(PRM, 16)
            for r in range(reps):
                if r > 0 and mode == "full":
                    # repeat r's GEMM overwrites tables repeat r-1 reads:
                    # gate on all of r-1's gathers
                    for (q, kk), val in rep_end_vals[r - 1].items():
                        s.wait_ge(GS[q][kk], val)
                if mode not in ("gather", "gatheronly"):
                    _emit_gemm_dmas(s, r)
                if mode in ("gemm", "gatheronly"):
                    continue
                # one bulk result write per pass
                s.wait_ge(VCH, vch_after[r * NBAT + NA - 1])
                s.dma_start(
                    pa[:].rearrange("(t p) f -> p t f", p=128), res_sb[:]
                ).then_inc(BWPA, 16)
                s.wait_ge(VCH, vch_after[r * NBAT + NBAT - 1])
                s.dma_start(
                    out[:].rearrange("(t p) f -> p t f", p=128), res_sb[:]
                ).then_inc(BWOUT, 16)

        @block.tensor
        def _(t: bass.BassTensorEngine):
            if mode in ("gather", "gatheronly"):
                return
            t.wait_ge(PRM, 16 * 5)  # all param DMAs done (incl. W)
            for gci, (r, ci, h, c0, nt) in enumerate(GCH):
                t.wait_ge(XTL[gci % 2], 16 * (gci // 2 + 1))
                if gci >= 2:
                    t.wait_ge(YCP, gci - 1)  # psum bank parity free
                ins = None
                for ti in range(nt):
                    ins = t.matmul(
                        out=ps[:, (gci % 2) * CH_T * FOUT + ti * FOUT:
                               (gci % 2) * CH_T * FOUT + (ti + 1) * FOUT],
                        lhsT=xt_sb[:, gci % 2, ti * 128: (ti + 1) * 128],
                        rhs=w_sb[:],
                        start=True, stop=True,
                    )
                ins.then_inc(MMC, 1)

        @block.scalar
        def _(a: bass.BassScalarEngine):
            if mode in ("gather", "gatheronly"):
                return
            for gci, (r, ci, h, c0, nt) in enumerate(GCH):
                a.wait_ge(MMC, gci + 1)
                if gci >= 2:
                    a.wait_ge(YWR[gci % 2], ywr_after[gci - 2][gci % 2])
                base = (gci % 2) * CH_T * FOUT
                a.activation(
                    out=y_sb[:, gci % 2, :nt, :].rearrange("p c f -> p (c f)"),
                    in_=ps[:, base: base + nt * FOUT],
                    func=mybir.ActivationFunctionType.Copy,
                ).then_inc(YCP, 1)

        @block.vector
        def _(v: bass.BassVectorEngine):
            if mode in ("gemm", "gatheronly"):
                return
            vc = [0]

            def chained(fn):
                # serialize dependent same-engine DVE ops via a chain sem
                # (the DVE pipeline gives no same-engine RAW guarantee)
                if vc[0]:
                    v.wait_ge(VCH, vc[0])
                ins = fn()
                ins.then_inc(VCH, 1)
                vc[0] += 1
                return ins

            v.wait_ge(PRM, 16 * 5)
            for gbi, (r, bi, h, t0, gt, db, off) in enumerate(GBT):
                for (q, kk, val) in batch_vals[gbi]:
                    v.wait_ge(GS[q][kk], val)
                if bi == 0 and r > 0:
                    v.wait_ge(BWOUT, 16 * r)   # res_sb drained (prev repeat)
                if bi == NA:
                    v.wait_ge(BWPA, 16 * (r + 1))  # res_sb drained (pass A)
                    for (q, kk, val) in xmap_vals[r]:
                        v.wait_ge(GS[q][kk], val)
                buf = g_sb[:, gbi % NBUF, : db * gt, :].rearrange(
                    "p (g d) f -> p g d f", g=gt
                )
                res = res_sb[:, t0: t0 + gt, :]
                last = None
                cur = db
                while cur > 1:
                    half = cur // 2
                    hi = cur - half
                    last = chained(lambda half=half, hi=hi: v.tensor_tensor(
                        out=buf[:, :, 0:half, :],
                        in0=buf[:, :, 0:half, :],
                        in1=buf[:, :, hi: hi + half, :],
                        op=mybir.AluOpType.add,
                    ))
                    cur = hi
                if h == 1:
                    chained(lambda: v.tensor_tensor(
                        out=buf[:, :, 0, :],
                        in0=buf[:, :, 0, :],
                        in1=x_sb[:, t0: t0 + gt, :],
                        op=mybir.AluOpType.add,
                    ))
                    last = chained(lambda: v.tensor_tensor(
                        out=res,
                        in0=buf[:, :, 0, :],
                        in1=isdb_sb[:, t0: t0 + gt, None].to_broadcast(
                            [128, gt, FOUT]
                        ),
                        op=mybir.AluOpType.mult,
                    ))
                else:
                    # move the pass-A sum into the result buffer
                    last = chained(lambda: v.tensor_scalar_mul(
                        out=res, in0=buf[:, :, 0, :], scalar1=1.0
                    ))
                assert vc[0] == vch_after[gbi], (gbi, vc[0], vch_after[gbi])

    nc.compile()
    return nc


# --------------------------------- kernel -----------------------------------
def prepare(edges, features, W2, b2):
    """Build (nc, in_maps, assemble) for the given full inputs."""
    edges = np.asarray(edges)
    X = np.asarray(features, dtype=np.float32)
    W2 = np.asarray(W2, dtype=np.float32)
    b2 = np.asarray(b2, dtype=np.float32)
    src = edges[0].astype(np.int64)
    dst = edges[1].astype(np.int64)

    isd, batches, inmaps, outmaps = _build_schedule(src, dst)

    key = tuple((h, tuple(batches[h])) for h in (0, 1))
    if key not in _cache:
        _cache[key] = _build_program(batches)
    nc = _cache[key]

    # XT: [128, NCOL], column col(n) = isd[n] * X[n]; pad columns zero
    Xs = X * isd[:, None]
    XT = np.zeros((FIN, NCOL), dtype=np.float32)
    cols = np.arange(N) + 216 * (np.arange(N) >= 25000)
    XT[:, cols] = Xs.T

    in_maps = []
    for k in range(NCORES):
        m = dict(inmaps[k])
        m["xt"] = XT
        m["w"] = W2
        in_maps.append(m)

    def assemble(results):
        z = np.empty((N, FOUT), dtype=np.float32)
        for k in range(NCORES):
            om = outmaps[k]
            valid = om >= 0
            z[om[valid]] = results[k]["out"][valid]
        return z + b2[None, :]

    return nc, in_maps, assemble


def kernel(edges, features, W1, b1, W2, b2):
    nc, in_maps, assemble = prepare(edges, features, W2, b2)
    res = run_bass_kernel_spmd(nc, in_maps, list(range(NCORES)))
    return assemble(res.results)



# revision 3
# speedup vs baseline: 6.8647x; 6.8647x over previous
"""GCN (PyG GCNConv) forward on 8 Trainium2 NeuronCores.

Reference computes z = D^-1/2 (A+I) D^-1/2 (X @ W2) + b2  (conv1 is dead code,
its result is never used).

Strategy ("message-GEMM", 1D destination partition):
  * Host: compute degrees + symmetric normalization, fold isd[src] into X
    (bf16), partition messages (edges + self loops) by destination shard
    (8 cores x 6250 nodes), degree-sort dst slots, and materialize the
    per-message source columns as one dense bf16 operand
    xtm[128 feats, S slots] per core (padding columns are zero).
  * Device (identical program on all 8 cores, per-core data via in_maps):
    stream xtm sequentially (full HBM rate, no gather descriptors at all);
    for each batch of g dst tiles the degree layers are accumulated directly
    in PSUM by the tensor engine:
        psum[64, g*128] (+)= W2.T @ xtm[:, layer d columns]   d = 0..db-1
    so the segmented sum over incoming messages IS the matmul accumulation.
    One DVE op per batch scales by isd[dst] and writes the result slab.
  * Host: inverse-permute per-core outputs into global row order, + b2.
"""

import numpy as np

import concourse.bacc as bacc
import concourse.bass as bass
import concourse.mybir as mybir
from concourse.bass_utils import run_bass_kernel_spmd

# ---------------- problem constants (hardcoded per contract) ----------------
N = 50000          # nodes
FIN = 128          # input channels
FOUT = 64          # output channels
NCORES = 8
PER = N // NCORES  # 6250 dst nodes per core
TILES = 49         # ceil(PER/128)
PADN = TILES * 128  # 6272 padded dst slots per core

CHUNK = 4096       # xtm slots per DMA chunk (1 MB bf16)
NBANK = 8          # psum banks (512 f32 each) rotated across batches

_cache = {}

BF16 = mybir.dt.np(mybir.dt.bfloat16)


# ------------------------------ host schedule -------------------------------
def _build_schedule(src, dst):
    """Returns (isd, batches, percore, outmaps).

    batches: [(t0, g, db)] shared across cores (g in {1,2,4}, g*db % 4 == 0,
    so every degree layer of g*128 slots lies inside one 4096-slot chunk and
    every batch starts at a 512-slot boundary).
    percore[k]: dict(cols=int32[S] global source row per xtm column (N = zero
    row), isdb=[64, PADN] f32). outmaps[k]: slot -> global node id (-1 pad).
    """
    msrc = np.concatenate([src, np.arange(N, dtype=np.int64)])
    mdst = np.concatenate([dst, np.arange(N, dtype=np.int64)])
    deg = np.bincount(mdst, minlength=N)
    isd = (1.0 / np.sqrt(np.maximum(deg, 1))).astype(np.float32)

    core = mdst // PER
    dloc = mdst - core * PER

    pc = []
    Dmax = np.zeros(TILES, dtype=np.int64)
    for k in range(NCORES):
        sel = core == k
        d = dloc[sel]
        s = msrc[sel]
        cnt = np.bincount(d, minlength=PER)
        order = np.argsort(cnt, kind="stable")  # ascending degree
        pos = np.empty(PER, dtype=np.int64)
        pos[order] = np.arange(PER) + (PADN - PER)  # dummies at slots 0..21
        o2 = np.argsort(d, kind="stable")
        ds = d[o2]
        starts = np.searchsorted(ds, np.arange(PER))
        j = np.arange(ds.shape[0]) - starts[ds]  # rank within dst
        mpos = pos[ds]
        cntpad = np.zeros(PADN, dtype=np.int64)
        cntpad[pos] = cnt
        Dmax = np.maximum(Dmax, cntpad.reshape(TILES, 128).max(axis=1))
        pc.append(dict(pos=pos, tile=mpos // 128, pslot=mpos % 128,
                       j=j, src=s[o2]))

    # batches: consecutive tiles, g in {1,2,4}, db padded so g*db % 4 == 0
    batches = []
    t0 = 0
    while t0 < TILES:
        g = 1
        db = int(Dmax[t0])
        for gtry in (2, 4):
            if t0 + gtry > TILES:
                break
            nd = int(Dmax[t0:t0 + gtry].max())
            waste = nd * gtry - int(Dmax[t0:t0 + gtry].sum())
            if waste > max(2 * gtry, (nd * gtry) // 16):
                break
            g, db = gtry, nd
        while (g * db) % 4:
            db += 1
        batches.append((t0, g, db))
        t0 += g

    off = np.zeros(TILES, dtype=np.int64)   # xtm column offset of tile's batch
    t0of = np.zeros(TILES, dtype=np.int64)  # batch t0 of each tile
    gof = np.zeros(TILES, dtype=np.int64)   # batch g of each tile
    S = 0
    for (t0, g, db) in batches:
        off[t0:t0 + g] = S
        t0of[t0:t0 + g] = t0
        gof[t0:t0 + g] = g
        S += 128 * g * db

    percore = []
    outmaps = []
    for k in range(NCORES):
        e = pc[k]
        cols = np.full(S, N, dtype=np.int64)  # default: zero row
        t = e["tile"]
        lin = off[t] + e["j"] * (gof[t] * 128) + (t - t0of[t]) * 128 + e["pslot"]
        cols[lin] = e["src"]
        isdb = np.zeros(PADN, dtype=np.float32)
        isdb[e["pos"]] = isd[k * PER:(k + 1) * PER]
        isdb64 = np.ascontiguousarray(
            np.broadcast_to(isdb[None, :], (FOUT, PADN)))
        percore.append(dict(cols=cols.astype(np.int32), isdb=isdb64))
        om = np.full(PADN, -1, dtype=np.int64)
        om[e["pos"]] = np.arange(k * PER, (k + 1) * PER)
        outmaps.append(om)

    return isd, batches, percore, outmaps


# ------------------------------ device program ------------------------------
def _build_program(batches, reps=1):
    nc = bacc.Bacc("TRN2", debug=False)
    f32 = mybir.dt.float32
    bf16 = mybir.dt.bfloat16

    S = sum(128 * g * db for (_, g, db) in batches)
    NB = len(batches)
    NCH = -(-S // CHUNK)  # chunks per rep

    xtm = nc.declare_dram_parameter("xtm", [FIN, S], bf16, isOutput=False)
    w = nc.declare_dram_parameter("w", [FIN, FOUT], bf16, isOutput=False)
    isdb = nc.declare_dram_parameter("isdb", [FOUT, PADN], f32, isOutput=False)
    out = nc.declare_dram_parameter("out", [FOUT, PADN], f32, isOutput=True)

    # window list (one matmul per degree layer; never crosses a chunk)
    wins = []  # (chunk, rhs_off_in_chunk, psum_off, cols, start, stop, batch)
    col = 0
    for b, (t0, g, db) in enumerate(batches):
        W = g * 128
        for d in range(db):
            c = col // CHUNK
            assert col // CHUNK == (col + W - 1) // CHUNK
            wins.append((c, col % CHUNK, (b % NBANK) * 512, W,
                         d == 0, d == db - 1, b))
            col += W
    NW = len(wins)
    assert col == S

    # per-batch cumulative matmul counts (rep-local)
    mm_after = [0] * NB
    for wi, (_, _, _, _, _, _, b) in enumerate(wins):
        mm_after[b] = wi + 1
    # last window index touching each chunk
    wlast = [0] * NCH
    for wi, (c, _, _, _, _, _, _) in enumerate(wins):
        wlast[c] = wi + 1

    from contextlib import ExitStack
    with ExitStack() as ctx:
        w_sb = ctx.enter_context(nc.sbuf_tensor("w_sb", [FIN, FOUT], bf16))
        xtm_sb = ctx.enter_context(
            nc.sbuf_tensor("xtm_sb", [FIN, 2, CHUNK], bf16))
        isdb_sb = ctx.enter_context(
            nc.sbuf_tensor("isdb_sb", [FOUT, PADN], f32))
        resf = ctx.enter_context(nc.sbuf_tensor("resf", [FOUT, PADN], f32))
        ps = ctx.enter_context(nc.psum_tensor("ps", [FOUT, NBANK * 512], f32))
        names = ["PRM", "XTL", "MMC", "VCH", "BWOUT"]
        sem = {n: ctx.enter_context(nc.semaphore(n)) for n in names}
        PRM, XTL, MMC, VCH, BWOUT = (sem[n] for n in names)
        block = ctx.enter_context(nc.Block())

        @block.sync
        def _(s: bass.BassEngine):
            s.dma_start(w_sb[:], w[:]).then_inc(PRM, 16)
            s.dma_start(isdb_sb[:], isdb[:]).then_inc(PRM, 16)
            for r in range(reps):
                for c in range(NCH):
                    gc = r * NCH + c
                    if gc >= 2:
                        pr, pcc = divmod(gc - 2, NCH)
                        s.wait_ge(MMC, pr * NW + wlast[pcc])
                    n = min(CHUNK, S - c * CHUNK)
                    s.dma_start(
                        xtm_sb[:, gc % 2, :n],
                        xtm[:, c * CHUNK: c * CHUNK + n],
                    ).then_inc(XTL, 16)
                s.wait_ge(VCH, (r + 1) * NB)
                s.dma_start(out[:], resf[:]).then_inc(BWOUT, 16)

        @block.tensor
        def _(t: bass.BassTensorEngine):
            t.wait_ge(PRM, 32)
            for r in range(reps):
                for wi, (c, co, po, W, st, sp, b) in enumerate(wins):
                    gb = r * NB + b
                    if st:
                        if gb >= NBANK:
                            t.wait_ge(VCH, gb - NBANK + 1)
                        t.wait_ge(XTL, 16 * (r * NCH + c + 1))
                    elif co == 0:
                        # first window of a new chunk mid-batch
                        t.wait_ge(XTL, 16 * (r * NCH + c + 1))
                    t.matmul(
                        out=ps[:, po: po + W],
                        lhsT=w_sb[:],
                        rhs=xtm_sb[:, (r * NCH + c) % 2, co: co + W],
                        start=st, stop=sp,
                    ).then_inc(MMC, 1)

        @block.vector
        def _(v: bass.BassVectorEngine):
            v.wait_ge(PRM, 32)
            for r in range(reps):
                for b, (t0, g, db) in enumerate(batches):
                    gb = r * NB + b
                    v.wait_ge(MMC, r * NW + mm_after[b])
                    if b == 0 and r > 0:
                        v.wait_ge(BWOUT, 16 * r)
                    v.tensor_tensor(
                        out=resf[:, t0 * 128: (t0 + g) * 128],
                        in0=ps[:, (gb % NBANK) * 512: (gb % NBANK) * 512 + g * 128],
                        in1=isdb_sb[:, t0 * 128: (t0 + g) * 128],
                        op=mybir.AluOpType.mult,
                    ).then_inc(VCH, 1)

    nc.compile()
    return nc


# --------------------------------- kernel -----------------------------------
def prepare(edges, features, W2, b2):
    """Build (nc, in_maps, assemble) for the given full inputs."""
    edges = np.asarray(edges)
    X = np.asarray(features, dtype=np.float32)
    W2 = np.asarray(W2, dtype=np.float32)
    b2 = np.asarray(b2, dtype=np.float32)
    src = edges[0].astype(np.int64)
    dst = edges[1].astype(np.int64)

    isd, batches, percore, outmaps = _build_schedule(src, dst)

    key = tuple(batches)
    if key not in _cache:
        _cache[key] = _build_program(batches)
    nc = _cache[key]

    # XpT: [128, N+1] bf16, col n = isd[n] * X[n]; col N is zero
    XpT = np.zeros((FIN, N + 1), dtype=BF16)
    XpT[:, :N] = (X * isd[:, None]).T.astype(BF16)
    Wb = W2.astype(BF16)

    in_maps = []
    for k in range(NCORES):
        in_maps.append(dict(
            xtm=np.ascontiguousarray(XpT[:, percore[k]["cols"]]),
            w=Wb,
            isdb=percore[k]["isdb"],
        ))

    def assemble(results):
        z = np.empty((N, FOUT), dtype=np.float32)
        for k in range(NCORES):
            om = outmaps[k]
            valid = om >= 0
            z[om[valid]] = results[k]["out"].T[valid]
        return z + b2[None, :]

    return nc, in_maps, assemble


def kernel(edges, features, W1, b1, W2, b2):
    nc, in_maps, assemble = prepare(edges, features, W2, b2)
    res = run_bass_kernel_spmd(nc, in_maps, list(range(NCORES)))
    return assemble(res.results)


# revision 7
# speedup vs baseline: 45.3523x; 6.6066x over previous
"""GCN (PyG GCNConv) forward on 8 Trainium2 NeuronCores.

Reference computes z = D^-1/2 (A+I) D^-1/2 (X @ W2) + b2  (conv1 is dead code,
its result is never used).

Strategy ("message-GEMM", 1D destination partition):
  * Host: compute degrees + symmetric normalization, fold isd[src] into X
    (bf16), partition messages (edges + self loops) by destination shard
    (8 cores x 6250 nodes), degree-sort dst slots, and materialize the
    per-message source columns as one dense bf16 operand
    xtm[128 feats, S slots] per core (padding columns are zero).
  * Device (identical program on all 8 cores, per-core data via in_maps):
    stream xtm sequentially (full HBM rate, no gather descriptors at all);
    for each batch of g dst tiles the degree layers are accumulated directly
    in PSUM by the tensor engine:
        psum[64, g*128] (+)= W2.T @ xtm[:, layer d columns]   d = 0..db-1
    so the segmented sum over incoming messages IS the matmul accumulation.
    One DVE op per batch scales by isd[dst] and writes the result slab.
  * Host: inverse-permute per-core outputs into global row order, + b2.
"""

import numpy as np

import concourse.bacc as bacc
import concourse.bass as bass
import concourse.mybir as mybir
from concourse.bass_utils import run_bass_kernel_spmd

# ---------------- problem constants (hardcoded per contract) ----------------
N = 50000          # nodes
FIN = 128          # input channels
FOUT = 64          # output channels
NCORES = 8
PER = N // NCORES  # 6250 dst nodes per core
TILES = 49         # ceil(PER/128)
PADN = TILES * 128  # 6272 padded dst slots per core

CHUNK = 4096       # xtm slots per DMA chunk (1 MB bf16)
NBANK = 8          # psum banks (512 f32 each) rotated across batches

_cache = {}

BF16 = mybir.dt.np(mybir.dt.bfloat16)


# ------------------------------ host schedule -------------------------------
def _build_schedule(src, dst):
    """Returns (isd, batches, percore, outmaps).

    batches: [(t0, g, db)] shared across cores (g in {1,2,4}, g*db % 4 == 0,
    so every degree layer of g*128 slots lies inside one 4096-slot chunk and
    every batch starts at a 512-slot boundary).
    percore[k]: dict(cols=int32[S] global source row per xtm column (N = zero
    row), isdb=[64, PADN] f32). outmaps[k]: slot -> global node id (-1 pad).
    """
    msrc = np.concatenate([src, np.arange(N, dtype=np.int64)])
    mdst = np.concatenate([dst, np.arange(N, dtype=np.int64)])
    deg = np.bincount(mdst, minlength=N)
    isd = (1.0 / np.sqrt(np.maximum(deg, 1))).astype(np.float32)

    core = mdst // PER
    dloc = mdst - core * PER

    pc = []
    Dmax = np.zeros(TILES, dtype=np.int64)
    for k in range(NCORES):
        sel = core == k
        d = dloc[sel]
        s = msrc[sel]
        cnt = np.bincount(d, minlength=PER)
        order = np.argsort(cnt, kind="stable")  # ascending degree
        pos = np.empty(PER, dtype=np.int64)
        pos[order] = np.arange(PER) + (PADN - PER)  # dummies at slots 0..21
        o2 = np.argsort(d, kind="stable")
        ds = d[o2]
        starts = np.searchsorted(ds, np.arange(PER))
        j = np.arange(ds.shape[0]) - starts[ds]  # rank within dst
        mpos = pos[ds]
        cntpad = np.zeros(PADN, dtype=np.int64)
        cntpad[pos] = cnt
        Dmax = np.maximum(Dmax, cntpad.reshape(TILES, 128).max(axis=1))
        pc.append(dict(pos=pos, tile=mpos // 128, pslot=mpos % 128,
                       j=j, src=s[o2]))

    # batches: consecutive tiles, g in {1,2,4}, db padded so g*db % 4 == 0
    batches = []
    t0 = 0
    while t0 < TILES:
        g = 1
        db = int(Dmax[t0])
        for gtry in (2, 4):
            if t0 + gtry > TILES:
                break
            nd = int(Dmax[t0:t0 + gtry].max())
            waste = nd * gtry - int(Dmax[t0:t0 + gtry].sum())
            if waste > max(2 * gtry, (nd * gtry) // 16):
                break
            g, db = gtry, nd
        while (g * db) % 4:
            db += 1
        batches.append((t0, g, db))
        t0 += g

    off = np.zeros(TILES, dtype=np.int64)   # xtm column offset of tile's batch
    t0of = np.zeros(TILES, dtype=np.int64)  # batch t0 of each tile
    gof = np.zeros(TILES, dtype=np.int64)   # batch g of each tile
    S = 0
    for (t0, g, db) in batches:
        off[t0:t0 + g] = S
        t0of[t0:t0 + g] = t0
        gof[t0:t0 + g] = g
        S += 128 * g * db

    percore = []
    outmaps = []
    for k in range(NCORES):
        e = pc[k]
        cols = np.full(S, N, dtype=np.int64)  # default: zero row
        t = e["tile"]
        lin = off[t] + e["j"] * (gof[t] * 128) + (t - t0of[t]) * 128 + e["pslot"]
        cols[lin] = e["src"]
        isdb = np.zeros(PADN, dtype=np.float32)
        isdb[e["pos"]] = isd[k * PER:(k + 1) * PER]
        isdb64 = np.ascontiguousarray(
            np.broadcast_to(isdb[None, :], (FOUT, PADN)))
        percore.append(dict(cols=cols.astype(np.int32), isdb=isdb64))
        om = np.full(PADN, -1, dtype=np.int64)
        om[e["pos"]] = np.arange(k * PER, (k + 1) * PER)
        outmaps.append(om)

    return isd, batches, percore, outmaps


# ------------------------------ device program ------------------------------
def _build_program(batches, reps=1):
    nc = bacc.Bacc("TRN2", debug=False)
    f32 = mybir.dt.float32
    bf16 = mybir.dt.bfloat16

    S = sum(128 * g * db for (_, g, db) in batches)
    NB = len(batches)
    NCH = -(-S // CHUNK)  # chunks per rep

    xtm = nc.declare_dram_parameter("xtm", [FIN, S], bf16, isOutput=False)
    w = nc.declare_dram_parameter("w", [FIN, FOUT], bf16, isOutput=False)
    isdb = nc.declare_dram_parameter("isdb", [FOUT, PADN], f32, isOutput=False)
    out = nc.declare_dram_parameter("out", [FOUT, PADN], f32, isOutput=True)

    # window list (one matmul per degree layer; never crosses a chunk)
    wins = []  # (chunk, rhs_off_in_chunk, psum_off, cols, start, stop, batch)
    col = 0
    for b, (t0, g, db) in enumerate(batches):
        W = g * 128
        for d in range(db):
            c = col // CHUNK
            assert col // CHUNK == (col + W - 1) // CHUNK
            wins.append((c, col % CHUNK, (b % NBANK) * 512, W,
                         d == 0, d == db - 1, b))
            col += W
    NW = len(wins)
    assert col == S

    # per-batch cumulative matmul counts (rep-local)
    mm_after = [0] * NB
    for wi, (_, _, _, _, _, _, b) in enumerate(wins):
        mm_after[b] = wi + 1
    # last window index touching each chunk
    wlast = [0] * NCH
    for wi, (c, _, _, _, _, _, _) in enumerate(wins):
        wlast[c] = wi + 1

    # output written in two pieces so most of the tail hides under the
    # last batches; piece 0 covers batches [0, NBH), piece 1 the rest
    NBH = NB // 2
    tile_h = batches[NBH][0]  # first tile of piece 1

    NBUF = 4  # chunk buffer rotation depth

    from contextlib import ExitStack
    with ExitStack() as ctx:
        w_sb = ctx.enter_context(nc.sbuf_tensor("w_sb", [FIN, FOUT], bf16))
        xtm_sb = ctx.enter_context(
            nc.sbuf_tensor("xtm_sb", [FIN, NBUF, CHUNK], bf16))
        isdb_sb = ctx.enter_context(
            nc.sbuf_tensor("isdb_sb", [FOUT, PADN], f32))
        resf = ctx.enter_context(nc.sbuf_tensor("resf", [FOUT, PADN], f32))
        ps = ctx.enter_context(nc.psum_tensor("ps", [FOUT, NBANK * 512], f32))
        names = ["PRMW", "PRMI", "XTL", "MMC", "VCH", "BWOUT"]
        sem = {n: ctx.enter_context(nc.semaphore(n)) for n in names}
        PRMW, PRMI, XTL, MMC, VCH, BWOUT = (sem[n] for n in names)
        block = ctx.enter_context(nc.Block())

        @block.sync
        def _(s: bass.BassEngine):
            first = [True]

            def load(c, r):
                gc = r * NCH + c
                if gc >= NBUF:
                    pr, pcc = divmod(gc - NBUF, NCH)
                    s.wait_ge(MMC, pr * NW + wlast[pcc])
                n = min(CHUNK, S - c * CHUNK)
                s.dma_start(
                    xtm_sb[:, gc % NBUF, :n],
                    xtm[:, c * CHUNK: c * CHUNK + n],
                ).then_inc(XTL, 16)

            for r in range(reps):
                for c in range(NCH):
                    load(c, r)
                    if first[0]:
                        # params slotted behind the first chunks so the
                        # tensor engine can start as early as possible
                        if c == 0:
                            s.dma_start(w_sb[:], w[:]).then_inc(PRMW, 16)
                        elif c == 1:
                            s.dma_start(isdb_sb[:], isdb[:]).then_inc(PRMI, 16)
                            first[0] = False
                s.wait_ge(VCH, (r + 1) * NB)
                s.dma_start(out[:], resf[:]).then_inc(BWOUT, 16)
            s.wait_ge(BWOUT, 16 * reps)

        @block.tensor
        def _(t: bass.BassTensorEngine):
            t.wait_ge(PRMW, 16)
            for r in range(reps):
                for wi, (c, co, po, W, st, sp, b) in enumerate(wins):
                    gb = r * NB + b
                    if st:
                        if gb >= NBANK:
                            t.wait_ge(VCH, gb - NBANK + 1)
                        t.wait_ge(XTL, 16 * (r * NCH + c + 1))
                    elif co == 0:
                        # first window of a new chunk mid-batch
                        t.wait_ge(XTL, 16 * (r * NCH + c + 1))
                    t.matmul(
                        out=ps[:, po: po + W],
                        lhsT=w_sb[:],
                        rhs=xtm_sb[:, (r * NCH + c) % NBUF, co: co + W],
                        start=st, stop=sp,
                    ).then_inc(MMC, 1)

        @block.vector
        def _(v: bass.BassVectorEngine):
            v.wait_ge(PRMI, 16)
            for r in range(reps):
                for b, (t0, g, db) in enumerate(batches):
                    gb = r * NB + b
                    v.wait_ge(MMC, r * NW + mm_after[b])
                    if r > 0 and b == 0:
                        # resf drained by the previous repeat's write
                        v.wait_ge(BWOUT, 16 * r)
                    v.tensor_tensor(
                        out=resf[:, t0 * 128: (t0 + g) * 128],
                        in0=ps[:, (gb % NBANK) * 512: (gb % NBANK) * 512 + g * 128],
                        in1=isdb_sb[:, t0 * 128: (t0 + g) * 128],
                        op=mybir.AluOpType.mult,
                    )
                    # sem inc via drain: a DVE op's own then_inc can fire
                    # before its SBUF writes are visible to other engines
                    v.drain().then_inc(VCH, 1)

    nc.compile()
    return nc


# --------------------------------- kernel -----------------------------------
def prepare(edges, features, W2, b2):
    """Build (nc, in_maps, assemble) for the given full inputs."""
    edges = np.asarray(edges)
    X = np.asarray(features, dtype=np.float32)
    W2 = np.asarray(W2, dtype=np.float32)
    b2 = np.asarray(b2, dtype=np.float32)
    src = edges[0].astype(np.int64)
    dst = edges[1].astype(np.int64)

    isd, batches, percore, outmaps = _build_schedule(src, dst)

    key = tuple(batches)
    if key not in _cache:
        _cache[key] = _build_program(batches)
    nc = _cache[key]

    # XpT: [128, N+1] bf16, col n = isd[n] * X[n]; col N is zero
    XpT = np.zeros((FIN, N + 1), dtype=BF16)
    XpT[:, :N] = (X * isd[:, None]).T.astype(BF16)
    Wb = W2.astype(BF16)

    in_maps = []
    for k in range(NCORES):
        in_maps.append(dict(
            xtm=np.ascontiguousarray(XpT[:, percore[k]["cols"]]),
            w=Wb,
            isdb=percore[k]["isdb"],
        ))

    def assemble(results):
        z = np.empty((N, FOUT), dtype=np.float32)
        for k in range(NCORES):
            om = outmaps[k]
            valid = om >= 0
            z[om[valid]] = results[k]["out"].T[valid]
        return z + b2[None, :]

    return nc, in_maps, assemble


def kernel(edges, features, W1, b1, W2, b2):
    nc, in_maps, assemble = prepare(edges, features, W2, b2)
    res = run_bass_kernel_spmd(nc, in_maps, list(range(NCORES)))
    return assemble(res.results)
